# revision 26
# baseline (speedup 1.0000x reference)
"""AlignBlock Trainium2 kernel — 8-core SPMD, no collectives.

Sharding: 8 cores = 2 batch x 4 time-chunks of 100 steps, fully independent
(halo-included input slices).

Device algorithm per core ("shifted K-conv variants", all-fp8 DoubleRow):
  The 5x3 conv over (t, d) of the QK^T scores is folded EXACTLY into the
  score matmul by pre-convolving the K projection with the 3 d-taps for each
  of the 5 time taps i:

      KG_i[k, y] = sum_j' wc[h,i,j'] * Kh[k, y + j' - 1]          (k = (h,f))
      Ck[x, j]   = sum_i sum_k Q[k, x+i-4] * KG_i[k, j+i-4]

  The Q-side time shifts are free SBUF column offsets of one shared Q buffer;
  the K-side shifts are baked into each variant's column layout. Q and KG
  both ship in fp8e4m3 (Q x8, KG x64 to clear the subnormal floor; the 1/512
  is undone by the exp activation's scale), which enables fp8 DoubleRow
  matmuls. The 5 variants' 16-row tail chunks are packed into ONE 80-row
  chunk (matching stacked-shift Q stationary), so kg ships 101 chunks.

  The additive softmax mask (band + exact d-edge leak corrections + conv
  bias, pre-scaled x512) enters the PSUM accumulation LAST (identity-weighted
  bf16 matmul), keeping the cm transfer off the critical start. Softmax is
  exp() on ACT straight out of PSUM; attention weights are transposed on the
  PE and applied to raw bf16 x_ref windows in column-split rounds so value
  matmuls pipeline against the tail of the xr DMA; 1/rowsum rides the
  PSUM->SBUF output copies.

  A warm-up burst of dummy matmuls at kernel start trips the PE HAM clock
  gate (1.2 -> 2.4 GHz) before the real convoy arrives. Transfers are
  ring-assigned in chunk-consumption order; output pieces stream out as
  their column groups finalize (one mid piece rides SWDGE/gpsimd).
"""

import numpy as np
import ml_dtypes

B, C, H, T, F, DELAY = 2, 16, 16, 400, 161, 100
TL = 100            # output timesteps per core
QT = 132            # mic-side cols (conv halo + DoubleRow M=128 padding)
NPAIR = 10          # DoubleRow chunk pairs per variant
QFW = NPAIR * 2 * QT + 128   # flat interleaved Q width + packed-tail block
KT = 203            # ref-side cols (window + conv halos)
NV = 5              # conv time taps = K variants
NCH = 20            # full 128-row chunks per variant
TOTCH = NV * NCH + 1         # 101 (incl. packed 80-row tail chunk)
KSCALE = 64.0       # fp8 pre-scale on KG
QSCALE = 8.0        # fp8 pre-scale on Q
NEG = -60.0         # out-of-band additive mask (pre-descale logits)
VB = [0, 493, 986, 1479, 1972, 2464, 2576]   # value/output chunks
XSPLIT = 1479       # xr column split (== VB[3]) for value-round pipelining

BF16 = ml_dtypes.bfloat16
FP8 = ml_dtypes.float8_e4m3

_CACHE = {}


def _build_raw():
    if "ncr" in _CACHE:
        return _CACHE["ncr"]
    import concourse.bass as bass
    from concourse import bacc, mybir

    dt = mybir.dt
    nc = bacc.Bacc("TRN2", target_bir_lowering=False, debug=False, num_devices=8)

    cm_d = nc.dram_tensor("cm", [128, 331], dt.bfloat16, kind="ExternalInput").ap()
    q_d = nc.dram_tensor("qf", [128, QFW], dt.float8e4, kind="ExternalInput").ap()
    kg_d = nc.dram_tensor("kg", [128, TOTCH, KT], dt.float8e4, kind="ExternalInput").ap()
    xr_d = nc.dram_tensor("xr", [KT, C * F], dt.bfloat16, kind="ExternalInput").ap()
    # two contiguous output tensors (per copy wave) so HBM writes are
    # unstrided; both issued from the otherwise-idle Sync engine
    out_d = [nc.dram_tensor("out0", [TL, VB[3]], dt.bfloat16,
                            kind="ExternalOutput").ap(),
             nc.dram_tensor("out1", [TL, C * F - VB[3]], dt.bfloat16,
                            kind="ExternalOutput").ap()]

    # static SBUF
    cmb = nc.alloc_sbuf_tensor("cmb", [128, 331], dt.bfloat16).ap()
    qb = nc.alloc_sbuf_tensor("qb", [128, QFW], dt.float8e4).ap()
    kgb = nc.alloc_sbuf_tensor("kgb", [128, TOTCH, KT], dt.float8e4).ap()
    xr01 = nc.alloc_sbuf_tensor("xr01", [128, 2, C * F], dt.bfloat16).ap()
    eb = nc.alloc_sbuf_tensor("eb", [TL, KT], dt.bfloat16).ap()
    ssum = nc.alloc_sbuf_tensor("ssum", [TL, 1], dt.float32).ap()
    rinv = nc.alloc_sbuf_tensor("rinv", [TL, 1], dt.float32).ap()
    a0 = nc.alloc_sbuf_tensor("a0", [128, TL], dt.bfloat16).ap()
    a1 = nc.alloc_sbuf_tensor("a1", [KT - 128, TL], dt.bfloat16).ap()
    ob = nc.alloc_sbuf_tensor("ob", [TL, C * F], dt.bfloat16).ap()
    warm = nc.alloc_sbuf_tensor("warm", [1, 2], dt.float32).ap()
    wsrc = nc.alloc_sbuf_tensor("wsrc", [128, 320], dt.float8e4).ap()

    # PSUM: 8 banks = ckb (scores; tp1 rides the same bank via bitcast,
    # temporally after exp consumed the scores) + tp0 + 6 value banks
    ckb = nc.alloc_psum_tensor("ckb", [128, 512], dt.float32).ap()
    ck = ckb[:, 0:KT]
    tp1 = ckb.bitcast(dt.bfloat16)[:, 512:612]     # bytes 1024.. (clear of ck)
    tp0 = nc.alloc_psum_tensor("tp0", [128, TL], dt.bfloat16).ap()
    po = [nc.alloc_psum_tensor(f"po{i}", [TL, 493], dt.float32).ap()
          for i in range(6)]

    identb = cmb[:, 0:128]
    maskb = cmb[:, 128:331]
    AF = mybir.ActivationFunctionType
    DR = mybir.MatmulPerfMode.DoubleRowSwInterleave
    from contextlib import ExitStack

    with ExitStack() as stack:
        block = stack.enter_context(nc.Block(no_gpsimd_drain=True))
        names = ["cmsem", "sQ", "sK1", "sK2", "sK3", "sK4", "sK5", "sK6",
                 "sxA1", "sxA2", "sxB1", "sxB2", "tsem", "esem", "tpsem",
                 "asem", "a1sem", "rsem", "pub", "cqv", "cqs", "odsem", "wsem"]
        sem = {n: stack.enter_context(nc.semaphore(n)) for n in names}
        (cmsem, sQ, sK1, sK2, sK3, sK4, sK5, sK6, sxA1, sxA2, sxB1, sxB2,
         tsem, esem, tpsem, asem, a1sem, rsem, pub, cqv, cqs, odsem, wsem) = (
            sem[n] for n in names)
        kwait = {0: sK1, 20: sK2, 40: sK3, 60: sK4}

        @block.sync
        def _(sync):
            sync.dma_start(out=qb[:], in_=q_d[:]).then_inc(sQ, 16)
            sync.dma_start(out=kgb[:, 20:40, :], in_=kg_d[:, 20:40, :]).then_inc(sK2, 16)
            sync.dma_start(out=kgb[:, 60:80, :], in_=kg_d[:, 60:80, :]).then_inc(sK4, 16)
            sync.dma_start(out=kgb[:, 93:101, :], in_=kg_d[:, 93:101, :]).then_inc(sK6, 16)
            sync.dma_start(out=xr01[0:KT - 128, 1, 0:XSPLIT],
                           in_=xr_d[128:KT, 0:XSPLIT]).then_inc(sxB1, 16)
            sync.dma_start(out=xr01[0:KT - 128, 1, XSPLIT:],
                           in_=xr_d[128:KT, XSPLIT:]).then_inc(sxB2, 16)
            sync.wait_ge(cqv, 2)
            sync.wait_ge(cqs, 1)
            sync.dma_start(out=out_d[0][:],
                           in_=ob[:, 0:VB[3]]).then_inc(odsem, 16)
            sync.wait_ge(cqv, 3)
            sync.wait_ge(cqs, 3)
            sync.dma_start(out=out_d[1][:],
                           in_=ob[:, VB[3]:]).then_inc(odsem, 16)
            sync.wait_ge(odsem, 32)

        @block.scalar
        def _(scalar):
            # pre-load the exp + copy activation tables while DMA ramps
            scalar.wait_ge(wsem, 1)
            scalar.activation(warm[:, 0:1], warm[:, 0:1], AF.Exp)
            scalar.copy(warm[:, 1:2], warm[:, 1:2])
            scalar.dma_start(out=kgb[:, 0:20, :], in_=kg_d[:, 0:20, :]).then_inc(sK1, 16)
            scalar.dma_start(out=cmb[:], in_=cm_d[:]).then_inc(cmsem, 16)
            scalar.dma_start(out=kgb[:, 40:60, :], in_=kg_d[:, 40:60, :]).then_inc(sK3, 16)
            scalar.dma_start(out=kgb[:, 80:93, :], in_=kg_d[:, 80:93, :]).then_inc(sK5, 16)
            scalar.dma_start(out=xr01[:, 0, 0:XSPLIT],
                             in_=xr_d[0:128, 0:XSPLIT]).then_inc(sxA1, 16)
            scalar.dma_start(out=xr01[:, 0, XSPLIT:],
                             in_=xr_d[0:128, XSPLIT:]).then_inc(sxA2, 16)
            # softmax exp straight off PSUM (descale by 1/(QSCALE*KSCALE)),
            # split so transposes start early
            scalar.wait_ge(tsem, 1)
            scalar.activation(eb[:, 0:128], ck[0:TL, 0:128], AF.Exp,
                              bias=0.0, scale=1.0 / (QSCALE * KSCALE)).then_inc(esem, 1)
            scalar.activation(eb[:, 128:KT], ck[0:TL, 128:KT], AF.Exp,
                              bias=0.0, scale=1.0 / (QSCALE * KSCALE)).then_inc(esem, 1)
            # attention-weight transpose copy (lower part)
            scalar.wait_ge(tpsem, 2)
            scalar.copy(a1[:], tp1[0:KT - 128, :]).then_inc(a1sem, 1)
            # output copies: 1/rowsum folded into scale
            scalar.wait_ge(pub, 3)
            scalar.wait_ge(rsem, 2)
            scalar.activation(ob[:, VB[1]:VB[2]], po[1][:],
                              AF.Copy, bias=0.0, scale=rinv[:]).then_inc(cqs, 1)
            scalar.wait_ge(pub, 6)
            scalar.activation(ob[:, VB[3]:VB[4]], po[3][:],
                              AF.Copy, bias=0.0, scale=rinv[:]).then_inc(cqs, 1)
            scalar.activation(ob[:, VB[5]:VB[6]], po[5][:, 0:VB[6] - VB[5]],
                              AF.Copy, bias=0.0, scale=rinv[:]).then_inc(cqs, 1)

        @block.tensor
        def _(tensor):
            # HAM warm-up: dummy matmuls on scratch while input DMA streams;
            # results land in the score bank and are cleared by start=True.
            for _w in range(28):
                tensor.matmul(ckb[:, 0:192], wsrc[:, 0:128], wsrc[:, 128:320],
                              start=True, stop=True)
            tensor.wait_ge(sQ, 16)

            def pair(cc, i, c, stop=False):
                # interleaved-reversed pair window: shift i -> even
                # element offset 2*(4-i) into the pair's 264-col block
                o = (c // 2) * 2 * QT + 2 * (4 - i)
                tensor.matmul(ck[:, :], qb[:, o:o + 256],
                              kgb[:, cc:cc + 2, :], start=(cc == 0),
                              stop=stop, perf_mode=DR)

            # kg chunk order: [0:80] variants 0-3, [80] packed tail,
            # [81:101] variant 4 — so the packed tail and the mask enter the
            # accumulation BEFORE the last kg group and exp fires right after
            # the final DR pair.
            for cc in range(0, 80, 2):
                if cc in kwait:
                    tensor.wait_ge(kwait[cc], 16)
                pair(cc, cc // NCH, cc % NCH)
            tensor.wait_ge(sK5, 16)
            # packed tail chunk: all 5 variants' 16-row tails at once,
            # against the stacked-shift Q stationary block
            tensor.matmul(ck[:, :], qb[:, NPAIR * 2 * QT:QFW], kgb[:, 80, :],
                          start=False, stop=False)
            for cc in range(81, 93, 2):
                pair(cc, 4, cc - 81)
            # mask + leak corrections + conv bias
            tensor.wait_ge(cmsem, 16)
            tensor.matmul(ck[:, :], identb[:, :], maskb[:, :],
                          start=False, stop=False)
            tensor.wait_ge(sK6, 16)
            for cc in range(93, 101, 2):
                pair(cc, 4, cc - 81, stop=(cc == 99))
            # drain fence publishes the finished score accumulation
            tensor.matmul(po[0][:, 0:128], kgb[:, 0, 0:TL], kgb[:, 0, 0:128],
                          start=True, stop=True).then_inc(tsem, 1)
            # transposes of attention weights, each published by its own
            # drain fence
            tensor.wait_ge(esem, 1)
            tensor.transpose(tp0[:], eb[:, 0:128], identb[0:TL, 0:TL])
            tensor.matmul(po[1][:, 0:128], kgb[:, 0, 0:TL], kgb[:, 0, 0:128],
                          start=True, stop=True).then_inc(tpsem, 1)
            tensor.wait_ge(esem, 2)
            tensor.transpose(tp1[0:KT - 128, :], eb[:, 128:KT], identb[0:TL, 0:TL])
            tensor.matmul(po[2][:, 0:128], kgb[:, 0, 0:TL], kgb[:, 0, 0:128],
                          start=True, stop=True).then_inc(tpsem, 1)
            # value matmuls in three bank-pair waves, each closed by a drain
            # fence, so copies and output pieces stream while later banks
            # still accumulate; xr piece waits line up with the DMA splits
            def va0(n):
                tensor.matmul(po[n][:, 0:VB[n + 1] - VB[n]], a0[:, :],
                              xr01[:, 0, VB[n]:VB[n + 1]], start=True, stop=False)

            def va1(n):
                tensor.matmul(po[n][:, 0:VB[n + 1] - VB[n]], a1[:, :],
                              xr01[0:KT - 128, 1, VB[n]:VB[n + 1]],
                              start=False, stop=True)

            tensor.wait_ge(asem, 1)
            tensor.wait_ge(sxA1, 16)
            va0(0)
            va0(1)
            va0(2)
            tensor.wait_ge(a1sem, 1)
            tensor.wait_ge(sxB1, 16)
            va1(0)
            va1(1)
            va1(2)
            tensor.matmul(ck[0:TL, 0:128], identb[:, 0:TL], identb[:, 0:128],
                          start=True, stop=True).then_inc(pub, 3)   # banks 0-2
            tensor.wait_ge(sxA2, 16)
            va0(3)
            va0(4)
            va0(5)
            tensor.wait_ge(sxB2, 16)
            va1(3)
            va1(4)
            va1(5)
            tensor.matmul(ck[0:TL, 0:128], identb[:, 0:TL], identb[:, 0:128],
                          start=True, stop=True).then_inc(pub, 3)   # banks 3-5

        @block.vector
        def _(vector):
            vector.memset(warm[:], 0.0).then_inc(wsem, 1)
            # attention-weight transpose copy (upper part)
            vector.wait_ge(tpsem, 1)
            vector.tensor_copy(a0[:], tp0[:]).then_inc(asem, 1)
            # row sums + reciprocal (tpsem>=2 implies both exp halves done)
            vector.wait_ge(tpsem, 2)
            vector.tensor_reduce(ssum[:], eb[:], axis=mybir.AxisListType.X,
                                 op=mybir.AluOpType.add).then_inc(rsem, 1)
            vector.wait_ge(rsem, 1)
            vector.reciprocal(rinv[:], ssum[:]).then_inc(rsem, 1)
            # output copies: even banks
            vector.wait_ge(rsem, 2)
            vector.wait_ge(pub, 3)
            vector.tensor_scalar_mul(ob[:, VB[0]:VB[1]], po[0][:],
                                     rinv[:]).then_inc(cqv, 1)
            vector.tensor_scalar_mul(ob[:, VB[2]:VB[3]], po[2][:],
                                     rinv[:]).then_inc(cqv, 1)
            vector.wait_ge(pub, 6)
            vector.tensor_scalar_mul(ob[:, VB[4]:VB[5]], po[4][:, 0:VB[5] - VB[4]],
                                     rinv[:]).then_inc(cqv, 1)

    nc.compile()
    _CACHE["ncr"] = nc
    return nc


def _host_prep(x_mic, x_ref, w_mic, b_mic, w_ref, b_ref, w_conv, b_conv):
    """Build the 8 per-core input maps (layout prep + tiny 1x1 projections)."""
    f32 = np.float32
    wc = w_conv[0]                                   # (H, 5, 3)
    Qh = np.einsum("hc,bctf->bhtf", w_mic, x_mic) + b_mic[None, :, None, None]
    Kh = np.einsum("hc,bctf->bhtf", w_ref, x_ref) + b_ref[None, :, None, None]
    PAD = 120
    Khp = np.pad(Kh, ((0, 0), (0, 0), (PAD, PAD), (0, 0)))
    Qhp = np.pad(Qh, ((0, 0), (0, 0), (8, 40), (0, 0)))
    xrp = np.pad(x_ref, ((0, 0), (0, 0), (PAD, PAD), (0, 0)))
    L = T + 2 * PAD
    # KGg[i][b,h,m,f] = sum_j' wc[h,i,j'] Khp[m + j'], tau(m) = m + 1 - PAD
    KGg = np.zeros((NV, B, H, L - 2, F), f32)
    for i in range(NV):
        for jp in range(3):
            KGg[i] += wc[:, i, jp][None, :, None, None] * Khp[:, :, jp:jp + L - 2, :]

    SC = QSCALE * KSCALE
    cm = np.zeros((128, 331), f32)
    cm[:, 0:128] = np.eye(128, dtype=f32)
    in_maps, core_meta = [], []
    for b in range(B):
        for tc in range(T // TL):
            t0 = tc * TL
            Qb = Qhp[b][:, t0 + 4:t0 + 4 + QT, :]            # x' in [-4, 128)
            qrows = Qb.transpose(0, 2, 1).reshape(H * F, QT) * QSCALE
            qp = np.zeros(((NCH + 1) * 128, QT), f32)
            qp[:H * F] = qrows
            qch = qp.reshape(NCH + 1, 128, QT).transpose(1, 0, 2)  # [128, 21, 132]
            qpack = np.zeros((128, QFW), f32)
            u = np.arange(QT)
            for p in range(NPAIR):
                qpack[:, p * 2 * QT + 2 * u] = qch[:, 2 * p, QT - 1 - u]
                qpack[:, p * 2 * QT + 2 * u + 1] = qch[:, 2 * p + 1, QT - 1 - u]
            # stacked-shift stationary for the packed 80-row tail chunk
            for i in range(NV):
                qpack[16 * i:16 * i + 16, NPAIR * 2 * QT:QFW] = \
                    qch[0:16, NCH, i:i + 128]
            qpack = np.ascontiguousarray(qpack).astype(FP8)
            # K variants, column-shifted so all matmuls read cols [0, KT)
            kgp = np.zeros((TOTCH, 128, KT), f32)
            for i in range(NV):
                m0 = t0 - 108 + i + PAD                      # tau = t0-107+i+j2
                sl = KGg[i, b][:, m0:m0 + KT, :]
                rows = sl.transpose(0, 2, 1).reshape(H * F, KT) * KSCALE
                tmp = np.zeros(((NCH + 1) * 128, KT), f32)
                tmp[:H * F] = rows
                tch = tmp.reshape(NCH + 1, 128, KT)
                # chunk order: variants 0-3 at [0:80], packed tail at [80],
                # variant 4 at [81:101]
                base = i * NCH if i < 4 else 81
                kgp[base:base + NCH] = tch[0:NCH]
                kgp[80][16 * i:16 * i + 16] = tch[NCH][0:16]
            kgpack = np.ascontiguousarray(kgp.transpose(1, 0, 2)).astype(FP8)
            # additive mask: band + exact d-edge leak corrections + conv bias
            x_idx = np.arange(TL)[:, None]
            j_idx = np.arange(KT)[None, :]
            band = (j_idx >= x_idx + 4) & (j_idx <= x_idx + 103)
            mask = np.where(band, 0.0, NEG).astype(f32)
            xs = np.arange(-4, TL)
            Qbl = Qb[:, 0:104, :]
            Dm1 = np.einsum("hxf,hxf->hx", Qbl, Khp[b][:, t0 + xs - 100 + PAD, :])
            Dp1 = np.einsum("hxf,hxf->hx", Qbl, Khp[b][:, t0 + xs + 1 + PAD, :])
            xv = np.arange(TL)
            leak0 = np.zeros(TL, f32)
            leak99 = np.zeros(TL, f32)
            for i in range(NV):
                leak0 += wc[:, i, 0] @ Dm1[:, xv + i]
                leak99 += wc[:, i, 2] @ Dp1[:, xv + i]
            mask[xv, xv + 4] -= leak0
            mask[xv, xv + 103] -= leak99
            mask += float(np.asarray(b_conv).reshape(-1)[0])
            cmc = cm.copy()
            cmc[:TL, 128:331] = mask * SC      # descaled together with scores
            # raw x_ref windows for the value matmul: [j, (c, f)]
            jt = t0 - 103 + np.arange(KT)
            xrw = xrp[b][:, jt + PAD, :].transpose(1, 0, 2).reshape(KT, C * F)
            xrb = np.ascontiguousarray(xrw).astype(BF16)
            in_maps.append({
                "cm": cmc.astype(BF16), "qf": qpack, "kg": kgpack, "xr": xrb,
            })
            core_meta.append((b, t0))
    return in_maps, core_meta


def kernel(**inputs):
    x_mic = np.asarray(inputs["x_mic"], dtype=np.float32)
    x_ref = np.asarray(inputs["x_ref"], dtype=np.float32)
    w_mic = np.asarray(inputs["w_mic"], dtype=np.float32)
    b_mic = np.asarray(inputs["b_mic"], dtype=np.float32)
    w_ref = np.asarray(inputs["w_ref"], dtype=np.float32)
    b_ref = np.asarray(inputs["b_ref"], dtype=np.float32)
    w_conv = np.asarray(inputs["w_conv"], dtype=np.float32)
    b_conv = np.asarray(inputs["b_conv"], dtype=np.float32)
    delay = int(inputs["delay"])
    assert delay == DELAY, f"kernel hardcodes delay={DELAY}, got {delay}"

    in_maps, core_meta = _host_prep(
        x_mic, x_ref, w_mic, b_mic, w_ref, b_ref, w_conv, b_conv
    )
    nc = _build_raw()
    from concourse.bass_utils import run_bass_kernel_spmd

    res = run_bass_kernel_spmd(nc, in_maps, core_ids=list(range(8)))
    out = np.zeros((B, C, T, F), dtype=np.float32)
    for (b, t0), r in zip(core_meta, res.results):
        o = np.concatenate(
            [np.asarray(r[f"out{i}"], dtype=np.float32) for i in range(2)],
            axis=1).reshape(TL, C, F)
        out[b, :, t0:t0 + TL, :] = o.transpose(1, 0, 2)
    return out


if __name__ == "__main__":
    z = np.load("/tmp/inputs.npz")
    ins = {k: z[k] for k in z.files}
    out = kernel(**ins)
    ref = np.load("/tmp/ref.npy")
    rel = np.abs(out - ref).max() / np.abs(ref).max()
    print("Relative error:", rel)


# revision 35
# speedup vs baseline: 1.1236x; 1.1236x over previous
"""AlignBlock Trainium2 kernel — 8-core SPMD, no collectives.

Sharding: 8 cores = 2 batch x 4 time-chunks of 100 steps, fully independent
(halo-included input slices).

Device algorithm per core ("shifted K-conv variants", all-fp8 DoubleRow):
  The 5x3 conv over (t, d) of the QK^T scores is folded EXACTLY into the
  score matmul by pre-convolving the K projection with the 3 d-taps for each
  of the 5 time taps i:

      KG_i[k, y] = sum_j' wc[h,i,j'] * Kh[k, y + j' - 1]          (k = (h,f))
      Ck[x, j]   = sum_i sum_k Q[k, x+i-4] * KG_i[k, j+i-4]

  The Q-side time shifts are free SBUF column offsets of one shared Q buffer;
  the K-side shifts are baked into each variant's column layout. Q and KG
  both ship in fp8e4m3 (Q x8, KG x64 to clear the subnormal floor; the 1/512
  is undone by the exp activation's scale), which enables fp8 DoubleRow
  matmuls. The 5 variants' 16-row tail chunks are packed into ONE 80-row
  chunk (matching stacked-shift Q stationary), so kg ships 101 chunks.

  The additive softmax mask (band + exact d-edge leak corrections + conv
  bias, pre-scaled x512) enters the PSUM accumulation LAST (identity-weighted
  bf16 matmul), keeping the cm transfer off the critical start. Softmax is
  exp() on ACT straight out of PSUM; attention weights are transposed on the
  PE and applied to raw bf16 x_ref windows in column-split rounds so value
  matmuls pipeline against the tail of the xr DMA; 1/rowsum rides the
  PSUM->SBUF output copies.

  A warm-up burst of dummy matmuls at kernel start trips the PE HAM clock
  gate (1.2 -> 2.4 GHz) before the real convoy arrives. Transfers are
  ring-assigned in chunk-consumption order; output pieces stream out as
  their column groups finalize (one mid piece rides SWDGE/gpsimd).
"""

import numpy as np
import ml_dtypes

B, C, H, T, F, DELAY = 2, 16, 16, 400, 161, 100
TL = 100            # output timesteps per core
QT = 132            # mic-side cols (conv halo + DoubleRow M=128 padding)
NPAIR = 10          # DoubleRow chunk pairs per variant
QFW = NPAIR * 2 * QT + 128   # flat interleaved Q width + packed-tail block
KT = 203            # ref-side cols (window + conv halos)
NV = 5              # conv time taps = K variants
NCH = 20            # full 128-row chunks per variant
TOTCH = NV * NCH + 1         # 101 (incl. packed 80-row tail chunk)
KSCALE = 64.0       # fp8 pre-scale on KG
QSCALE = 8.0        # fp8 pre-scale on Q
NEG = -60.0         # out-of-band additive mask (pre-descale logits)
VB = [0, 493, 986, 1479, 1972, 2464, 2576]   # value/output chunks
XSPLIT = 1479       # xr column split (== VB[3]) for value-round pipelining

BF16 = ml_dtypes.bfloat16
FP8 = ml_dtypes.float8_e4m3

_CACHE = {}


def _build_raw():
    if "ncr" in _CACHE:
        return _CACHE["ncr"]
    import concourse.bass as bass
    from concourse import bacc, mybir

    dt = mybir.dt
    nc = bacc.Bacc("TRN2", target_bir_lowering=False, debug=False, num_devices=8)

    # qf and cm ship as ONE byte-tensor: the SDMA engines round-robin between
    # the two HWDGE rings at PACKET (per-partition-run) granularity, so a
    # small-packet transfer (cm alone: 662B) starves its whole ring while the
    # other ring streams 4KB kg packets. Merged: 3430B packets.
    QCMW = QFW + 331 * 2
    qcm_d = nc.dram_tensor("qcm", [128, QCMW], dt.uint8, kind="ExternalInput").ap()
    kg_d = nc.dram_tensor("kg", [128, TOTCH, KT], dt.float8e4, kind="ExternalInput").ap()
    xr_d = nc.dram_tensor("xr", [KT, C * F], dt.bfloat16, kind="ExternalInput").ap()
    # two contiguous output tensors (per copy wave) so HBM writes are
    # unstrided; both issued from the otherwise-idle Sync engine
    out_d = [nc.dram_tensor("out0", [TL, VB[3]], dt.bfloat16,
                            kind="ExternalOutput").ap(),
             nc.dram_tensor("out1", [TL, C * F - VB[3]], dt.bfloat16,
                            kind="ExternalOutput").ap()]

    # static SBUF (qf + cm share one allocation; typed views below)
    qcmb = nc.alloc_sbuf_tensor("qcmb", [128, QCMW], dt.uint8).ap()
    qb = qcmb.bitcast(dt.float8e4)[:, 0:QFW]
    cmb = qcmb.bitcast(dt.bfloat16)[:, QFW // 2:QCMW // 2]
    kgb = nc.alloc_sbuf_tensor("kgb", [128, TOTCH, KT], dt.float8e4).ap()
    xr01 = nc.alloc_sbuf_tensor("xr01", [128, 2, C * F], dt.bfloat16).ap()
    eb = nc.alloc_sbuf_tensor("eb", [TL, KT], dt.bfloat16).ap()
    ssum = nc.alloc_sbuf_tensor("ssum", [TL, 1], dt.float32).ap()
    rinv = nc.alloc_sbuf_tensor("rinv", [TL, 1], dt.float32).ap()
    a0 = nc.alloc_sbuf_tensor("a0", [128, TL], dt.bfloat16).ap()
    a1 = nc.alloc_sbuf_tensor("a1", [KT - 128, TL], dt.bfloat16).ap()
    ob = nc.alloc_sbuf_tensor("ob", [TL, C * F], dt.bfloat16).ap()
    warm = nc.alloc_sbuf_tensor("warm", [1, 2], dt.float32).ap()
    wsrc = nc.alloc_sbuf_tensor("wsrc", [128, 320], dt.float8e4).ap()

    # PSUM: 8 banks = ckb (scores; tp1 rides the same bank via bitcast,
    # temporally after exp consumed the scores) + tp0 + 6 value banks
    ckb = nc.alloc_psum_tensor("ckb", [128, 512], dt.float32).ap()
    ck = ckb[:, 0:KT]
    tp1 = ckb.bitcast(dt.bfloat16)[:, 512:612]     # bytes 1024.. (clear of ck)
    tp0 = nc.alloc_psum_tensor("tp0", [128, TL], dt.bfloat16).ap()
    po = [nc.alloc_psum_tensor(f"po{i}", [TL, 493], dt.float32).ap()
          for i in range(6)]

    identb = cmb[:, 0:128]
    maskb = cmb[:, 128:331]
    AF = mybir.ActivationFunctionType
    DR = mybir.MatmulPerfMode.DoubleRowSwInterleave
    from contextlib import ExitStack

    with ExitStack() as stack:
        block = stack.enter_context(nc.Block(no_gpsimd_drain=True))
        names = ["sQ", "sK1", "sK2", "sK3", "sK4", "sK5", "sK6",
                 "sxA1", "sxA2", "sxB1", "sxB2", "tsem", "esem", "tpsem",
                 "asem", "a1sem", "rsem", "pub", "cqv", "cqs", "odsem", "wsem"]
        sem = {n: stack.enter_context(nc.semaphore(n)) for n in names}
        (sQ, sK1, sK2, sK3, sK4, sK5, sK6, sxA1, sxA2, sxB1, sxB2,
         tsem, esem, tpsem, asem, a1sem, rsem, pub, cqv, cqs, odsem, wsem) = (
            sem[n] for n in names)
        kwait = {0: sK1, 20: sK2, 40: sK3, 60: sK4}

        @block.sync
        def _(sync):
            sync.dma_start(out=qcmb[:], in_=qcm_d[:]).then_inc(sQ, 16)
            sync.dma_start(out=kgb[:, 20:40, :], in_=kg_d[:, 20:40, :]).then_inc(sK2, 16)
            sync.dma_start(out=kgb[:, 60:80, :], in_=kg_d[:, 60:80, :]).then_inc(sK4, 16)
            sync.dma_start(out=kgb[:, 93:101, :], in_=kg_d[:, 93:101, :]).then_inc(sK6, 16)
            sync.dma_start(out=xr01[0:KT - 128, 1, 0:XSPLIT],
                           in_=xr_d[128:KT, 0:XSPLIT]).then_inc(sxB1, 16)
            sync.dma_start(out=xr01[0:KT - 128, 1, XSPLIT:],
                           in_=xr_d[128:KT, XSPLIT:]).then_inc(sxB2, 16)
            sync.wait_ge(cqv, 2)
            sync.wait_ge(cqs, 1)
            sync.dma_start(out=out_d[0][:],
                           in_=ob[:, 0:VB[3]]).then_inc(odsem, 16)
            sync.wait_ge(odsem, 32)

        @block.scalar
        def _(scalar):
            # pre-load the exp + copy activation tables while DMA ramps
            scalar.wait_ge(wsem, 1)
            scalar.activation(warm[:, 0:1], warm[:, 0:1], AF.Exp)
            scalar.copy(warm[:, 1:2], warm[:, 1:2])
            scalar.dma_start(out=kgb[:, 0:20, :], in_=kg_d[:, 0:20, :]).then_inc(sK1, 16)
            scalar.dma_start(out=kgb[:, 40:60, :], in_=kg_d[:, 40:60, :]).then_inc(sK3, 16)
            scalar.dma_start(out=kgb[:, 80:93, :], in_=kg_d[:, 80:93, :]).then_inc(sK5, 16)
            scalar.dma_start(out=xr01[:, 0, 0:XSPLIT],
                             in_=xr_d[0:128, 0:XSPLIT]).then_inc(sxA1, 16)
            scalar.dma_start(out=xr01[:, 0, XSPLIT:],
                             in_=xr_d[0:128, XSPLIT:]).then_inc(sxA2, 16)
            # softmax exp straight off PSUM (descale by 1/(QSCALE*KSCALE)),
            # split so transposes start early
            scalar.wait_ge(tsem, 1)
            scalar.activation(eb[:, 0:128], ck[0:TL, 0:128], AF.Exp,
                              bias=0.0, scale=1.0 / (QSCALE * KSCALE)).then_inc(esem, 1)
            scalar.activation(eb[:, 128:KT], ck[0:TL, 128:KT], AF.Exp,
                              bias=0.0, scale=1.0 / (QSCALE * KSCALE)).then_inc(esem, 1)
            # attention-weight transpose copy (lower part)
            scalar.wait_ge(tpsem, 2)
            scalar.copy(a1[:], tp1[0:KT - 128, :]).then_inc(a1sem, 1)
            # output copies: 1/rowsum folded into scale
            scalar.wait_ge(pub, 3)
            scalar.wait_ge(rsem, 2)
            scalar.activation(ob[:, VB[1]:VB[2]], po[1][:],
                              AF.Copy, bias=0.0, scale=rinv[:]).then_inc(cqs, 1)
            scalar.wait_ge(pub, 6)
            scalar.activation(ob[:, VB[3]:VB[4]], po[3][:],
                              AF.Copy, bias=0.0, scale=rinv[:]).then_inc(cqs, 1)
            scalar.activation(ob[:, VB[5]:VB[6]], po[5][:, 0:VB[6] - VB[5]],
                              AF.Copy, bias=0.0, scale=rinv[:]).then_inc(cqs, 1)
            scalar.wait_ge(cqv, 3)
            scalar.dma_start(out=out_d[1][:],
                             in_=ob[:, VB[3]:]).then_inc(odsem, 16)

        @block.tensor
        def _(tensor):
            # HAM warm-up: dummy matmuls on scratch while input DMA streams;
            # results land in the score bank and are cleared by start=True.
            for _w in range(28):
                tensor.matmul(ckb[:, 0:192], wsrc[:, 0:128], wsrc[:, 128:320],
                              start=True, stop=True)
            tensor.wait_ge(sQ, 16)

            def pair(cc, i, c, stop=False):
                # interleaved-reversed pair window: shift i -> even
                # element offset 2*(4-i) into the pair's 264-col block
                o = (c // 2) * 2 * QT + 2 * (4 - i)
                tensor.matmul(ck[:, :], qb[:, o:o + 256],
                              kgb[:, cc:cc + 2, :], start=(cc == 0),
                              stop=stop, perf_mode=DR)

            # kg chunk order: [0:80] variants 0-3, [80] packed tail,
            # [81:101] variant 4 — so the packed tail and the mask enter the
            # accumulation BEFORE the last kg group and exp fires right after
            # the final DR pair.
            for cc in range(0, 80, 2):
                if cc in kwait:
                    tensor.wait_ge(kwait[cc], 16)
                pair(cc, cc // NCH, cc % NCH)
            tensor.wait_ge(sK5, 16)
            # packed tail chunk: all 5 variants' 16-row tails at once,
            # against the stacked-shift Q stationary block
            tensor.matmul(ck[:, :], qb[:, NPAIR * 2 * QT:QFW], kgb[:, 80, :],
                          start=False, stop=False)
            for cc in range(81, 93, 2):
                pair(cc, 4, cc - 81)
            # mask + leak corrections + conv bias (cmb rode the qcm transfer,
            # already covered by the sQ wait)
            tensor.matmul(ck[:, :], identb[:, :], maskb[:, :],
                          start=False, stop=False)
            tensor.wait_ge(sK6, 16)
            for cc in range(93, 101, 2):
                pair(cc, 4, cc - 81, stop=(cc == 99))
            # drain fence publishes the finished score accumulation
            tensor.matmul(po[0][:, 0:128], kgb[:, 0, 0:TL], kgb[:, 0, 0:128],
                          start=True, stop=True).then_inc(tsem, 1)
            # transposes of attention weights, each published by its own
            # drain fence
            tensor.wait_ge(esem, 1)
            tensor.transpose(tp0[:], eb[:, 0:128], identb[0:TL, 0:TL])
            tensor.matmul(po[1][:, 0:128], kgb[:, 0, 0:TL], kgb[:, 0, 0:128],
                          start=True, stop=True).then_inc(tpsem, 1)
            tensor.wait_ge(esem, 2)
            tensor.transpose(tp1[0:KT - 128, :], eb[:, 128:KT], identb[0:TL, 0:TL])
            tensor.matmul(po[2][:, 0:128], kgb[:, 0, 0:TL], kgb[:, 0, 0:128],
                          start=True, stop=True).then_inc(tpsem, 1)
            # value matmuls in three bank-pair waves, each closed by a drain
            # fence, so copies and output pieces stream while later banks
            # still accumulate; xr piece waits line up with the DMA splits
            def va0(n):
                tensor.matmul(po[n][:, 0:VB[n + 1] - VB[n]], a0[:, :],
                              xr01[:, 0, VB[n]:VB[n + 1]], start=True, stop=False)

            def va1(n):
                tensor.matmul(po[n][:, 0:VB[n + 1] - VB[n]], a1[:, :],
                              xr01[0:KT - 128, 1, VB[n]:VB[n + 1]],
                              start=False, stop=True)

            tensor.wait_ge(asem, 1)
            tensor.wait_ge(sxA1, 16)
            va0(0)
            va0(1)
            va0(2)
            tensor.wait_ge(a1sem, 1)
            tensor.wait_ge(sxB1, 16)
            va1(0)
            va1(1)
            va1(2)
            tensor.matmul(ck[0:TL, 0:128], identb[:, 0:TL], identb[:, 0:128],
                          start=True, stop=True).then_inc(pub, 3)   # banks 0-2
            tensor.wait_ge(sxA2, 16)
            va0(3)
            va0(4)
            va0(5)
            tensor.wait_ge(sxB2, 16)
            va1(3)
            va1(4)
            va1(5)
            tensor.matmul(ck[0:TL, 0:128], identb[:, 0:TL], identb[:, 0:128],
                          start=True, stop=True).then_inc(pub, 3)   # banks 3-5

        @block.vector
        def _(vector):
            vector.memset(warm[:], 0.0).then_inc(wsem, 1)
            # attention-weight transpose copy (upper part)
            vector.wait_ge(tpsem, 1)
            vector.tensor_copy(a0[:], tp0[:]).then_inc(asem, 1)
            # row sums + reciprocal (tpsem>=2 implies both exp halves done)
            vector.wait_ge(tpsem, 2)
            vector.tensor_reduce(ssum[:], eb[:], axis=mybir.AxisListType.X,
                                 op=mybir.AluOpType.add).then_inc(rsem, 1)
            vector.wait_ge(rsem, 1)
            vector.reciprocal(rinv[:], ssum[:]).then_inc(rsem, 1)
            # output copies: even banks
            vector.wait_ge(rsem, 2)
            vector.wait_ge(pub, 3)
            vector.tensor_scalar_mul(ob[:, VB[0]:VB[1]], po[0][:],
                                     rinv[:]).then_inc(cqv, 1)
            vector.tensor_scalar_mul(ob[:, VB[2]:VB[3]], po[2][:],
                                     rinv[:]).then_inc(cqv, 1)
            vector.wait_ge(pub, 6)
            vector.tensor_scalar_mul(ob[:, VB[4]:VB[5]], po[4][:, 0:VB[5] - VB[4]],
                                     rinv[:]).then_inc(cqv, 1)

    nc.compile()
    _CACHE["ncr"] = nc
    return nc


def _host_prep(x_mic, x_ref, w_mic, b_mic, w_ref, b_ref, w_conv, b_conv):
    """Build the 8 per-core input maps (layout prep + tiny 1x1 projections)."""
    f32 = np.float32
    wc = w_conv[0]                                   # (H, 5, 3)
    Qh = np.einsum("hc,bctf->bhtf", w_mic, x_mic) + b_mic[None, :, None, None]
    Kh = np.einsum("hc,bctf->bhtf", w_ref, x_ref) + b_ref[None, :, None, None]
    PAD = 120
    Khp = np.pad(Kh, ((0, 0), (0, 0), (PAD, PAD), (0, 0)))
    Qhp = np.pad(Qh, ((0, 0), (0, 0), (8, 40), (0, 0)))
    xrp = np.pad(x_ref, ((0, 0), (0, 0), (PAD, PAD), (0, 0)))
    L = T + 2 * PAD
    # KGg[i][b,h,m,f] = sum_j' wc[h,i,j'] Khp[m + j'], tau(m) = m + 1 - PAD
    KGg = np.zeros((NV, B, H, L - 2, F), f32)
    for i in range(NV):
        for jp in range(3):
            KGg[i] += wc[:, i, jp][None, :, None, None] * Khp[:, :, jp:jp + L - 2, :]

    SC = QSCALE * KSCALE
    cm = np.zeros((128, 331), f32)
    cm[:, 0:128] = np.eye(128, dtype=f32)
    in_maps, core_meta = [], []
    for b in range(B):
        for tc in range(T // TL):
            t0 = tc * TL
            Qb = Qhp[b][:, t0 + 4:t0 + 4 + QT, :]            # x' in [-4, 128)
            qrows = Qb.transpose(0, 2, 1).reshape(H * F, QT) * QSCALE
            qp = np.zeros(((NCH + 1) * 128, QT), f32)
            qp[:H * F] = qrows
            qch = qp.reshape(NCH + 1, 128, QT).transpose(1, 0, 2)  # [128, 21, 132]
            qpack = np.zeros((128, QFW), f32)
            u = np.arange(QT)
            for p in range(NPAIR):
                qpack[:, p * 2 * QT + 2 * u] = qch[:, 2 * p, QT - 1 - u]
                qpack[:, p * 2 * QT + 2 * u + 1] = qch[:, 2 * p + 1, QT - 1 - u]
            # stacked-shift stationary for the packed 80-row tail chunk
            for i in range(NV):
                qpack[16 * i:16 * i + 16, NPAIR * 2 * QT:QFW] = \
                    qch[0:16, NCH, i:i + 128]
            qpack = np.ascontiguousarray(qpack).astype(FP8)
            # K variants, column-shifted so all matmuls read cols [0, KT)
            kgp = np.zeros((TOTCH, 128, KT), f32)
            for i in range(NV):
                m0 = t0 - 108 + i + PAD                      # tau = t0-107+i+j2
                sl = KGg[i, b][:, m0:m0 + KT, :]
                rows = sl.transpose(0, 2, 1).reshape(H * F, KT) * KSCALE
                tmp = np.zeros(((NCH + 1) * 128, KT), f32)
                tmp[:H * F] = rows
                tch = tmp.reshape(NCH + 1, 128, KT)
                # chunk order: variants 0-3 at [0:80], packed tail at [80],
                # variant 4 at [81:101]
                base = i * NCH if i < 4 else 81
                kgp[base:base + NCH] = tch[0:NCH]
                kgp[80][16 * i:16 * i + 16] = tch[NCH][0:16]
            kgpack = np.ascontiguousarray(kgp.transpose(1, 0, 2)).astype(FP8)
            # additive mask: band + exact d-edge leak corrections + conv bias
            x_idx = np.arange(TL)[:, None]
            j_idx = np.arange(KT)[None, :]
            band = (j_idx >= x_idx + 4) & (j_idx <= x_idx + 103)
            mask = np.where(band, 0.0, NEG).astype(f32)
            xs = np.arange(-4, TL)
            Qbl = Qb[:, 0:104, :]
            Dm1 = np.einsum("hxf,hxf->hx", Qbl, Khp[b][:, t0 + xs - 100 + PAD, :])
            Dp1 = np.einsum("hxf,hxf->hx", Qbl, Khp[b][:, t0 + xs + 1 + PAD, :])
            xv = np.arange(TL)
            leak0 = np.zeros(TL, f32)
            leak99 = np.zeros(TL, f32)
            for i in range(NV):
                leak0 += wc[:, i, 0] @ Dm1[:, xv + i]
                leak99 += wc[:, i, 2] @ Dp1[:, xv + i]
            mask[xv, xv + 4] -= leak0
            mask[xv, xv + 103] -= leak99
            mask += float(np.asarray(b_conv).reshape(-1)[0])
            cmc = cm.copy()
            cmc[:TL, 128:331] = mask * SC      # descaled together with scores
            # raw x_ref windows for the value matmul: [j, (c, f)]
            jt = t0 - 103 + np.arange(KT)
            xrw = xrp[b][:, jt + PAD, :].transpose(1, 0, 2).reshape(KT, C * F)
            xrb = np.ascontiguousarray(xrw).astype(BF16)
            qcm = np.ascontiguousarray(np.concatenate(
                [qpack.view(np.uint8),
                 np.ascontiguousarray(cmc.astype(BF16)).view(np.uint8)], axis=1))
            in_maps.append({
                "qcm": qcm, "kg": kgpack, "xr": xrb,
            })
            core_meta.append((b, t0))
    return in_maps, core_meta


def kernel(**inputs):
    x_mic = np.asarray(inputs["x_mic"], dtype=np.float32)
    x_ref = np.asarray(inputs["x_ref"], dtype=np.float32)
    w_mic = np.asarray(inputs["w_mic"], dtype=np.float32)
    b_mic = np.asarray(inputs["b_mic"], dtype=np.float32)
    w_ref = np.asarray(inputs["w_ref"], dtype=np.float32)
    b_ref = np.asarray(inputs["b_ref"], dtype=np.float32)
    w_conv = np.asarray(inputs["w_conv"], dtype=np.float32)
    b_conv = np.asarray(inputs["b_conv"], dtype=np.float32)
    delay = int(inputs["delay"])
    assert delay == DELAY, f"kernel hardcodes delay={DELAY}, got {delay}"

    in_maps, core_meta = _host_prep(
        x_mic, x_ref, w_mic, b_mic, w_ref, b_ref, w_conv, b_conv
    )
    nc = _build_raw()
    from concourse.bass_utils import run_bass_kernel_spmd

    res = run_bass_kernel_spmd(nc, in_maps, core_ids=list(range(8)))
    out = np.zeros((B, C, T, F), dtype=np.float32)
    for (b, t0), r in zip(core_meta, res.results):
        o = np.concatenate(
            [np.asarray(r[f"out{i}"], dtype=np.float32) for i in range(2)],
            axis=1).reshape(TL, C, F)
        out[b, :, t0:t0 + TL, :] = o.transpose(1, 0, 2)
    return out


if __name__ == "__main__":
    z = np.load("/tmp/inputs.npz")
    ins = {k: z[k] for k in z.files}
    out = kernel(**ins)
    ref = np.load("/tmp/ref.npy")
    rel = np.abs(out - ref).max() / np.abs(ref).max()
    print("Relative error:", rel)
